# revision 25
# baseline (speedup 1.0000x reference)
"""Trainium2 Bass kernel for nn_CumulativeFlattenedLinear (segment_reduce).

Per window of S=64 timesteps: per-timestep C->O projection (weights zero for
the first n_discard steps) + causal cumsum within the window, plus bias.

Strategy (data-parallel over batch, 1 batch element per core):
  - The host marshals x to fp16, drops the n_discard zero-weight positions of
    every window (25% of the data), and lays it out pre-transposed as
    x^T[(c,v'), (st, w, u, tchunk)] so the device reads matmul stationary
    slabs directly: no on-chip shuffle, transpose, or PSUM round-trip.
  - Per window: 6 triangular "intra" matmuls write the window's (s,o)-major
    PSUM region; block totals accumulate into a shared 128-col PSUM "pre"
    region seeded with bias by a K=1 matmul (prefix sums via PSUM
    accumulation). One DVE broadcast-add per window pair evicts
    intra+prefix to fp16 output tiles.
  - Only s >= n_discard positions are stored (bias-only positions are filled
    host-side): 3.14MB in + 3.14MB out per core, multi-KB DMA runs, loads
    and stores split across both HWDGE queues at window-pair granularity.
"""
import numpy as np

import concourse.bass as bass
import concourse.tile as tile
from concourse import bacc, mybir
from concourse.bass_utils import run_bass_kernel_spmd

F16 = mybir.dt.float16
F32 = mybir.dt.float32

B, C, T, O = 8, 16, 131072, 16
P = 128
CH = 512                  # time elems per partition row per supertile
NST = T // (P * CH)       # 2 supertiles
V = 8                     # sub-block length
NU = 8                    # sub-blocks per window
S = NU * V                # 64
NW = CH // S              # windows per partition row = 8

_cache = {}


def _build_nc(first_u):
    DU = NU - first_u          # active sub-blocks (trailing)
    NPRE = DU - 1              # blocks contributing prefix totals
    KEEP = DU * V              # stored positions per window (s >= fill)
    NWIN = NST * NW            # 16 windows per partition row overall
    XW = DU * P                # xT cols per window
    YW = O * KEEP              # y cols per window
    PREB = DU * 128            # pre region offset within a window's psum

    nc = bacc.Bacc("TRN2", target_bir_lowering=False, debug=False)
    xt_d = nc.dram_tensor("xT", (P, NWIN * XW), F16, kind="ExternalInput")
    wi_d = nc.dram_tensor("w_intra", (P, DU * 128), F16, kind="ExternalInput")
    wp_d = nc.dram_tensor("w_pre", (P, max(NPRE, 1) * 128), F16,
                          kind="ExternalInput")
    ones_d = nc.dram_tensor("ones_k1", (1, P), F16, kind="ExternalInput")
    brow_d = nc.dram_tensor("biasrow", (1, P), F16, kind="ExternalInput")
    y_d = nc.dram_tensor("y", (P, NWIN * YW), F16, kind="ExternalOutput")

    with tile.TileContext(nc) as tc:
        with (
            tc.tile_pool(name="const", bufs=1) as cp,
            tc.tile_pool(name="xtp", bufs=1) as xtp,
            tc.tile_pool(name="outp", bufs=6) as outp,
            tc.tile_pool(name="mid", bufs=3) as mid,
            tc.tile_pool(name="psW", bufs=2, space="PSUM") as psW,
        ):
            # constants go through the GPSIMD SWDGE queue so they don't
            # delay the first x chunk on the HWDGE queues
            w_intra = cp.tile([P, DU * 128], F16, name="w_intra")
            nc.gpsimd.dma_start(w_intra[:], wi_d.ap())
            w_pre = cp.tile([P, max(NPRE, 1) * 128], F16, name="w_pre")
            nc.gpsimd.dma_start(w_pre[:], wp_d.ap())
            ones = cp.tile([1, P], F16, name="ones_k1")
            nc.gpsimd.dma_start(ones[:], ones_d.ap())
            brow = cp.tile([1, P], F16, name="biasrow")
            nc.gpsimd.dma_start(brow[:], brow_d.ap())

            # x^T loads at window-pair granularity, alternating HWDGE queues
            xt = xtp.tile([P, NWIN * XW], F16, name="xt")
            NP = NWIN // 2     # window pairs
            for pi in range(NP):
                eng = nc.sync if pi % 2 == 0 else nc.scalar
                eng.dma_start(xt[:, pi * 2 * XW:(pi + 1) * 2 * XW],
                              xt_d.ap()[:, pi * 2 * XW:(pi + 1) * 2 * XW])

            def win_mm(pw, half, wi):
                """All matmuls for window wi into pair-tile half."""
                base = half * (PREB + P)
                for du in range(DU):
                    nc.tensor.matmul(
                        pw[:, base + du * 128:base + (du + 1) * 128],
                        xt[:, wi * XW + du * 128:wi * XW + (du + 1) * 128],
                        w_intra[:, du * 128:(du + 1) * 128],
                        start=True, stop=True, skip_group_check=True,
                    )
                # bias seed AFTER intras: start=True clears has_written
                # bank-wide and the pre region shares banks with intra blocks
                nc.tensor.matmul(
                    pw[:, base + PREB:base + PREB + P], ones[:], brow[:],
                    start=True, stop=(NPRE == 0), skip_group_check=True,
                )
                for pu in range(NPRE):
                    nc.tensor.matmul(
                        pw[:, base + PREB:base + PREB + P],
                        xt[:, wi * XW + pu * 128:wi * XW + (pu + 1) * 128],
                        w_pre[:, pu * 128:(pu + 1) * 128],
                        start=False, stop=(pu == NPRE - 1),
                        skip_group_check=True,
                    )

            for pi in range(NP):
                pw = psW.tile([P, 2 * (PREB + P)], F32, name="pw", tag="pw")
                win_mm(pw, 0, 2 * pi)
                win_mm(pw, 1, 2 * pi + 1)
                # prefix totals to SBUF (DVE can't read 2 PSUM operands)
                pre_sb = mid.tile([P, 2 * P], F32, name="pre_sb", tag="pre")
                nc.scalar.copy(
                    pre_sb[:].rearrange("p (h x) -> p h x", h=2),
                    pw[:].rearrange("p (h x) -> p h x", h=2)[:, :, PREB:],
                )
                # eviction: out[(w, o, s)] = intra + prefix, fp16
                # (DVE ISA allows max 3 free AP dims: one op per window)
                ot = outp.tile([P, 2 * YW], F16, name="ot", tag="ot")
                for h in range(2):
                    out4 = ot[:, h * YW:(h + 1) * YW].rearrange(
                        "p (o u v) -> p o u v", o=O, u=DU, v=V)
                    in1 = pw[:, h * (PREB + P):h * (PREB + P) + PREB].rearrange(
                        "p (u v o) -> p o u v", u=DU, v=V, o=O)
                    in2 = pre_sb[:, h * P + first_u * O:(h + 1) * P].rearrange(
                        "p (u o) -> p o u", u=DU, o=O
                    ).unsqueeze(3).broadcast_to([P, O, DU, V])
                    nc.vector.tensor_add(out4, in1, in2)
                eng = nc.scalar if pi % 2 == 0 else nc.sync
                eng.dma_start(
                    y_d.ap()[:, pi * 2 * YW:(pi + 1) * 2 * YW], ot[:])
    nc.compile()
    return nc


def _host_constants(weight, bias, n_discard, n_keep):
    Swin = n_discard + n_keep
    assert Swin == S and n_discard % V == 0
    first_u = n_discard // V
    DU = NU - first_u
    NPRE = DU - 1

    w = weight.reshape(O, C, n_keep).transpose(2, 1, 0).astype(np.float32)
    w_full = np.concatenate(
        [np.zeros((n_discard, C, O), np.float32), w], axis=0
    )  # (S, C, O)

    # w_intra[k=(c,vp), du*128 + v*16 + o] = w_full[u*8+vp, c, o] if vp<=v
    blk = np.stack([w_full[(first_u + du) * V:(first_u + du + 1) * V]
                    for du in range(DU)])          # (DU, V, C, O)
    tri = np.zeros((DU, C, V, V, O), np.float32)   # (du, c, vp, v, o)
    vp = np.arange(V)
    for v in range(V):
        tri[:, :, vp <= v, v, :] = blk.transpose(0, 2, 1, 3)[:, :, vp <= v]
    w_intra = tri.reshape(DU, C * V, V * O).transpose(1, 0, 2).reshape(
        P, DU * 128)

    # w_pre[k=(c,vp), pu*128 + ut*16 + o] = w_full[u*8+vp, c, o] if ut>u
    pre = np.zeros((max(NPRE, 1), C, V, NU, O), np.float32)
    for pu in range(NPRE):
        u = first_u + pu
        pre[pu, :, :, u + 1:, :] = blk[pu].transpose(1, 0, 2)[:, :, None, :]
    w_pre = pre.reshape(max(NPRE, 1), C * V, NU * O).transpose(1, 0, 2).reshape(
        P, max(NPRE, 1) * 128)

    bias32 = bias.astype(np.float32)
    consts = {
        "w_intra": np.ascontiguousarray(w_intra).astype(np.float16),
        "w_pre": np.ascontiguousarray(w_pre).astype(np.float16),
        "ones_k1": np.ones((1, P), np.float16),
        "biasrow": np.tile(bias32, NU)[None, :].astype(np.float16),
    }
    return consts, first_u


def _run(inputs, trace=False):
    x = np.asarray(inputs["x"], dtype=np.float32)
    weight = np.asarray(inputs["weight"], dtype=np.float32)
    bias = np.asarray(inputs["bias"], dtype=np.float32)
    n_discard = int(inputs["n_discard"])
    n_keep = int(inputs["n_keep"])
    assert x.shape == (B, C, T) and weight.shape == (O, C * n_keep)

    consts, first_u = _host_constants(weight, bias, n_discard, n_keep)
    DU = NU - first_u
    key = ("nc", first_u)
    if key not in _cache:
        _cache[key] = _build_nc(first_u)
    nc = _cache[key]

    # marshal x: fp16, drop s < n_discard, pre-transpose to
    # [(c,v), (st, w, u, tchunk p)]
    x16 = x.astype(np.float16)
    xr = x16.reshape(B, C, NST, P, NW, NU, V)[:, :, :, :, :, first_u:, :]
    # -> (B, C, V, NST, NW, DU, P) -> (B, (c v), (st w u p))
    xT = np.ascontiguousarray(xr.transpose(0, 1, 6, 2, 4, 5, 3)).reshape(
        B, C * V, NST * NW * DU * P)

    in_maps = []
    for b in range(B):
        m = dict(consts)
        m["xT"] = xT[b]
        in_maps.append(m)
    res = run_bass_kernel_spmd(nc, in_maps, list(range(B)), trace=trace)

    # assemble full output: device gives s >= n_discard, host fills bias
    KEEP = DU * V
    y = np.empty((B, O, T), np.float32)
    yw = y.reshape(B, O, NST, P, NW, S)
    yw[:, :, :, :, :, :n_discard] = bias.astype(np.float32)[
        None, :, None, None, None, None]
    for b in range(B):
        yd = res.results[b]["y"].reshape(P, NST, NW, O, KEEP)
        yw[b, :, :, :, :, n_discard:] = yd.transpose(3, 1, 0, 2, 4)
    return y, res


def kernel(**inputs):
    y, _ = _run(inputs, trace=False)
    return y


# revision 27
# speedup vs baseline: 1.0525x; 1.0525x over previous
"""Trainium2 Bass kernel for nn_CumulativeFlattenedLinear (segment_reduce).

Per window of S=64 timesteps: per-timestep C->O projection (weights zero for
the first n_discard steps) + causal cumsum within the window, plus bias.

Strategy (data-parallel over batch, 1 batch element per core):
  - The host marshals x to fp16, drops the n_discard zero-weight positions of
    every window (25% of the data), and lays it out pre-transposed as
    x^T[(c,v'), (st, w, u, tchunk)] so the device reads matmul stationary
    slabs directly: no on-chip shuffle, transpose, or PSUM round-trip.
  - Per window: 6 triangular "intra" matmuls write the window's (s,o)-major
    PSUM region; block totals accumulate into a shared 128-col PSUM "pre"
    region seeded with bias by a K=1 matmul (prefix sums via PSUM
    accumulation). One DVE broadcast-add per window pair evicts
    intra+prefix to fp16 output tiles.
  - Only s >= n_discard positions are stored (bias-only positions are filled
    host-side): 3.14MB in + 3.14MB out per core, multi-KB DMA runs, loads
    and stores split across both HWDGE queues at window-pair granularity.
"""
import numpy as np

import concourse.bass as bass
import concourse.tile as tile
from concourse import bacc, mybir
from concourse.bass_utils import run_bass_kernel_spmd

F16 = mybir.dt.float16
F32 = mybir.dt.float32

B, C, T, O = 8, 16, 131072, 16
P = 128
CH = 512                  # time elems per partition row per supertile
NST = T // (P * CH)       # 2 supertiles
V = 8                     # sub-block length
NU = 8                    # sub-blocks per window
S = NU * V                # 64
NW = CH // S              # windows per partition row = 8

_cache = {}


def _build_nc(first_u):
    DU = NU - first_u          # active sub-blocks (trailing)
    NPRE = DU - 1              # blocks contributing prefix totals
    KEEP = DU * V              # stored positions per window (s >= fill)
    NWIN = NST * NW            # 16 windows per partition row overall
    XW = DU * P                # xT cols per window
    YW = O * KEEP              # y cols per window
    PREB = DU * 128            # pre region offset within a window's psum

    nc = bacc.Bacc("TRN2", target_bir_lowering=False, debug=False)
    xt_d = nc.dram_tensor("xT", (P, NWIN * XW), F16, kind="ExternalInput")
    wi_d = nc.dram_tensor("w_intra", (P, DU * 128), F16, kind="ExternalInput")
    wp_d = nc.dram_tensor("w_pre", (P, max(NPRE, 1) * 128), F16,
                          kind="ExternalInput")
    ones_d = nc.dram_tensor("ones_k1", (1, P), F16, kind="ExternalInput")
    brow_d = nc.dram_tensor("biasrow", (1, P), F16, kind="ExternalInput")
    y_d = nc.dram_tensor("y", (P, NWIN * YW), F16, kind="ExternalOutput")

    with tile.TileContext(nc) as tc:
        with (
            tc.tile_pool(name="const", bufs=1) as cp,
            tc.tile_pool(name="xtp", bufs=1) as xtp,
            tc.tile_pool(name="outp", bufs=6) as outp,
            tc.tile_pool(name="mid", bufs=3) as mid,
            tc.tile_pool(name="psW", bufs=2, space="PSUM") as psW,
        ):
            # constants split across both HWDGE queue heads (SWDGE is ~66GB/s)
            w_intra = cp.tile([P, DU * 128], F16, name="w_intra")
            nc.sync.dma_start(w_intra[:], wi_d.ap())
            w_pre = cp.tile([P, max(NPRE, 1) * 128], F16, name="w_pre")
            nc.scalar.dma_start(w_pre[:], wp_d.ap())
            ones = cp.tile([1, P], F16, name="ones_k1")
            nc.sync.dma_start(ones[:], ones_d.ap())
            brow = cp.tile([1, P], F16, name="biasrow")
            nc.scalar.dma_start(brow[:], brow_d.ap())

            # PE warm-up: ~3us of dummy matmuls on a memset tile while the
            # loads stream, so the PE clock is at full speed for pair 0
            warm_sb = cp.tile([P, 512], F16, name="warm_sb")
            nc.gpsimd.memset(warm_sb[:], 0)

            # x^T loads at window-pair granularity, alternating HWDGE queues
            xt = xtp.tile([P, NWIN * XW], F16, name="xt")
            NP = NWIN // 2     # window pairs
            for pi in range(NP):
                eng = nc.sync if pi % 2 == 0 else nc.scalar
                eng.dma_start(xt[:, pi * 2 * XW:(pi + 1) * 2 * XW],
                              xt_d.ap()[:, pi * 2 * XW:(pi + 1) * 2 * XW])

            warm_pw = psW.tile([P, 512], F32, name="warm_pw", tag="pw")
            for _ in range(8):
                nc.tensor.matmul(
                    warm_pw[:], warm_sb[:, 0:128], warm_sb[:],
                    start=True, stop=True, skip_group_check=True,
                )

            def win_mm(pw, half, wi):
                """All matmuls for window wi into pair-tile half."""
                base = half * (PREB + P)
                for du in range(DU):
                    nc.tensor.matmul(
                        pw[:, base + du * 128:base + (du + 1) * 128],
                        xt[:, wi * XW + du * 128:wi * XW + (du + 1) * 128],
                        w_intra[:, du * 128:(du + 1) * 128],
                        start=True, stop=True, skip_group_check=True,
                    )
                # bias seed AFTER intras: start=True clears has_written
                # bank-wide and the pre region shares banks with intra blocks
                nc.tensor.matmul(
                    pw[:, base + PREB:base + PREB + P], ones[:], brow[:],
                    start=True, stop=(NPRE == 0), skip_group_check=True,
                )
                for pu in range(NPRE):
                    nc.tensor.matmul(
                        pw[:, base + PREB:base + PREB + P],
                        xt[:, wi * XW + pu * 128:wi * XW + (pu + 1) * 128],
                        w_pre[:, pu * 128:(pu + 1) * 128],
                        start=False, stop=(pu == NPRE - 1),
                        skip_group_check=True,
                    )

            for pi in range(NP):
                pw = psW.tile([P, 2 * (PREB + P)], F32, name="pw", tag="pw")
                win_mm(pw, 0, 2 * pi)
                win_mm(pw, 1, 2 * pi + 1)
                # prefix totals to SBUF (DVE can't read 2 PSUM operands)
                pre_sb = mid.tile([P, 2 * P], F32, name="pre_sb", tag="pre")
                nc.scalar.copy(
                    pre_sb[:].rearrange("p (h x) -> p h x", h=2),
                    pw[:].rearrange("p (h x) -> p h x", h=2)[:, :, PREB:],
                )
                # eviction: out[(w, o, s)] = intra + prefix, fp16
                # (DVE ISA allows max 3 free AP dims: one op per window)
                ot = outp.tile([P, 2 * YW], F16, name="ot", tag="ot")
                for h in range(2):
                    out4 = ot[:, h * YW:(h + 1) * YW].rearrange(
                        "p (o u v) -> p o u v", o=O, u=DU, v=V)
                    in1 = pw[:, h * (PREB + P):h * (PREB + P) + PREB].rearrange(
                        "p (u v o) -> p o u v", u=DU, v=V, o=O)
                    in2 = pre_sb[:, h * P + first_u * O:(h + 1) * P].rearrange(
                        "p (u o) -> p o u", u=DU, o=O
                    ).unsqueeze(3).broadcast_to([P, O, DU, V])
                    nc.vector.tensor_add(out4, in1, in2)
                eng = nc.scalar if pi % 2 == 0 else nc.sync
                eng.dma_start(
                    y_d.ap()[:, pi * 2 * YW:(pi + 1) * 2 * YW], ot[:])
    nc.compile()
    return nc


def _host_constants(weight, bias, n_discard, n_keep):
    Swin = n_discard + n_keep
    assert Swin == S and n_discard % V == 0
    first_u = n_discard // V
    DU = NU - first_u
    NPRE = DU - 1

    w = weight.reshape(O, C, n_keep).transpose(2, 1, 0).astype(np.float32)
    w_full = np.concatenate(
        [np.zeros((n_discard, C, O), np.float32), w], axis=0
    )  # (S, C, O)

    # w_intra[k=(c,vp), du*128 + v*16 + o] = w_full[u*8+vp, c, o] if vp<=v
    blk = np.stack([w_full[(first_u + du) * V:(first_u + du + 1) * V]
                    for du in range(DU)])          # (DU, V, C, O)
    tri = np.zeros((DU, C, V, V, O), np.float32)   # (du, c, vp, v, o)
    vp = np.arange(V)
    for v in range(V):
        tri[:, :, vp <= v, v, :] = blk.transpose(0, 2, 1, 3)[:, :, vp <= v]
    w_intra = tri.reshape(DU, C * V, V * O).transpose(1, 0, 2).reshape(
        P, DU * 128)

    # w_pre[k=(c,vp), pu*128 + ut*16 + o] = w_full[u*8+vp, c, o] if ut>u
    pre = np.zeros((max(NPRE, 1), C, V, NU, O), np.float32)
    for pu in range(NPRE):
        u = first_u + pu
        pre[pu, :, :, u + 1:, :] = blk[pu].transpose(1, 0, 2)[:, :, None, :]
    w_pre = pre.reshape(max(NPRE, 1), C * V, NU * O).transpose(1, 0, 2).reshape(
        P, max(NPRE, 1) * 128)

    bias32 = bias.astype(np.float32)
    consts = {
        "w_intra": np.ascontiguousarray(w_intra).astype(np.float16),
        "w_pre": np.ascontiguousarray(w_pre).astype(np.float16),
        "ones_k1": np.ones((1, P), np.float16),
        "biasrow": np.tile(bias32, NU)[None, :].astype(np.float16),
    }
    return consts, first_u


def _run(inputs, trace=False):
    x = np.asarray(inputs["x"], dtype=np.float32)
    weight = np.asarray(inputs["weight"], dtype=np.float32)
    bias = np.asarray(inputs["bias"], dtype=np.float32)
    n_discard = int(inputs["n_discard"])
    n_keep = int(inputs["n_keep"])
    assert x.shape == (B, C, T) and weight.shape == (O, C * n_keep)

    consts, first_u = _host_constants(weight, bias, n_discard, n_keep)
    DU = NU - first_u
    key = ("nc", first_u)
    if key not in _cache:
        _cache[key] = _build_nc(first_u)
    nc = _cache[key]

    # marshal x: fp16, drop s < n_discard, pre-transpose to
    # [(c,v), (st, w, u, tchunk p)]
    x16 = x.astype(np.float16)
    xr = x16.reshape(B, C, NST, P, NW, NU, V)[:, :, :, :, :, first_u:, :]
    # -> (B, C, V, NST, NW, DU, P) -> (B, (c v), (st w u p))
    xT = np.ascontiguousarray(xr.transpose(0, 1, 6, 2, 4, 5, 3)).reshape(
        B, C * V, NST * NW * DU * P)

    in_maps = []
    for b in range(B):
        m = dict(consts)
        m["xT"] = xT[b]
        in_maps.append(m)
    res = run_bass_kernel_spmd(nc, in_maps, list(range(B)), trace=trace)

    # assemble full output: device gives s >= n_discard, host fills bias
    KEEP = DU * V
    y = np.empty((B, O, T), np.float32)
    yw = y.reshape(B, O, NST, P, NW, S)
    yw[:, :, :, :, :, :n_discard] = bias.astype(np.float32)[
        None, :, None, None, None, None]
    for b in range(B):
        yd = res.results[b]["y"].reshape(P, NST, NW, O, KEEP)
        yw[b, :, :, :, :, n_discard:] = yd.transpose(3, 1, 0, 2, 4)
    return y, res


def kernel(**inputs):
    y, _ = _run(inputs, trace=False)
    return y


# revision 29
# speedup vs baseline: 1.2072x; 1.1470x over previous
"""Trainium2 Bass kernel for nn_CumulativeFlattenedLinear (segment_reduce).

Per window of S=64 timesteps: per-timestep C->O projection (weights zero for
the first n_discard steps) + causal cumsum within the window, plus bias.

Strategy (data-parallel over batch, 1 batch element per core):
  - The host marshals x to fp16, drops the n_discard zero-weight positions of
    every window (25% of the data), and lays it out pre-transposed as
    x^T[(c,v'), (st, w, u, tchunk)] so the device reads matmul stationary
    slabs directly: no on-chip shuffle, transpose, or PSUM round-trip.
  - Per window: 6 triangular "intra" matmuls write the window's (s,o)-major
    PSUM region; block totals accumulate into a shared 128-col PSUM "pre"
    region seeded with bias by a K=1 matmul (prefix sums via PSUM
    accumulation). One DVE broadcast-add per window pair evicts
    intra+prefix to fp16 output tiles.
  - Only s >= n_discard positions are stored (bias-only positions are filled
    host-side): 3.14MB in + 3.14MB out per core, multi-KB DMA runs, loads
    and stores split across both HWDGE queues at window-pair granularity.
"""
import numpy as np

import concourse.bass as bass
import concourse.tile as tile
from concourse import bacc, mybir
from concourse.bass_utils import run_bass_kernel_spmd

F16 = mybir.dt.float16
F32 = mybir.dt.float32

B, C, T, O = 8, 16, 131072, 16
P = 128
CH = 512                  # time elems per partition row per supertile
NST = T // (P * CH)       # 2 supertiles
V = 8                     # sub-block length
NU = 8                    # sub-blocks per window
S = NU * V                # 64
NW = CH // S              # windows per partition row = 8

_cache = {}


def _build_nc(first_u):
    DU = NU - first_u          # active sub-blocks (trailing)
    NPRE = DU - 1              # blocks contributing prefix totals
    KEEP = DU * V              # stored positions per window (s >= fill)
    NWIN = NST * NW            # 16 windows per partition row overall
    XW = DU * P                # xT cols per window
    YW = O * KEEP              # y cols per window
    PREB = DU * 128            # pre region offset within a window's psum

    nc = bacc.Bacc("TRN2", target_bir_lowering=False, debug=False)
    xt_d = nc.dram_tensor("xT", (P, NWIN * XW), F16, kind="ExternalInput")
    wi_d = nc.dram_tensor("w_intra", (P, DU * 128), F16, kind="ExternalInput")
    wp_d = nc.dram_tensor("w_pre", (P, max(NPRE, 1) * 128), F16,
                          kind="ExternalInput")
    ones_d = nc.dram_tensor("ones_k1", (1, P), F16, kind="ExternalInput")
    brow_d = nc.dram_tensor("biasrow", (1, P), F16, kind="ExternalInput")
    y_d = nc.dram_tensor("y", (P, NWIN * YW), F16, kind="ExternalOutput")

    with tile.TileContext(nc) as tc:
        with (
            tc.tile_pool(name="const", bufs=1) as cp,
            tc.tile_pool(name="xtp", bufs=1) as xtp,
            tc.tile_pool(name="outp", bufs=6) as outp,
            tc.tile_pool(name="mid", bufs=3) as mid,
            tc.tile_pool(name="psW", bufs=2, space="PSUM") as psW,
        ):
            # constants split across both HWDGE queue heads (SWDGE is ~66GB/s)
            w_intra = cp.tile([P, DU * 128], F16, name="w_intra")
            nc.sync.dma_start(w_intra[:], wi_d.ap())
            w_pre = cp.tile([P, max(NPRE, 1) * 128], F16, name="w_pre")
            nc.scalar.dma_start(w_pre[:], wp_d.ap())
            ones = cp.tile([1, P], F16, name="ones_k1")
            nc.sync.dma_start(ones[:], ones_d.ap())
            brow = cp.tile([1, P], F16, name="biasrow")
            nc.scalar.dma_start(brow[:], brow_d.ap())

            # PE warm-up: ~3us of dummy matmuls on a memset tile while the
            # loads stream, so the PE clock is at full speed for pair 0
            warm_sb = cp.tile([P, 512], F16, name="warm_sb")
            nc.gpsimd.memset(warm_sb[:], 0)

            # x^T loads at window-pair granularity, alternating HWDGE queues
            xt = xtp.tile([P, NWIN * XW], F16, name="xt")
            NP = NWIN // 2     # window pairs
            for pi in range(NP):
                eng = nc.sync if pi % 2 == 0 else nc.scalar
                eng.dma_start(xt[:, pi * 2 * XW:(pi + 1) * 2 * XW],
                              xt_d.ap()[:, pi * 2 * XW:(pi + 1) * 2 * XW])

            warm_pw = psW.tile([P, 512], F32, name="warm_pw", tag="pw")
            for _ in range(6):
                nc.tensor.matmul(
                    warm_pw[:, 0:256], warm_sb[:, 0:128], warm_sb[:, 0:256],
                    start=True, stop=True, skip_group_check=True,
                )

            # pair psum layout: [w0 intra | w1 intra | w0 pre | w1 pre] so the
            # (h, u) axes merge and ONE 3-free-dim DVE op evicts both windows;
            # both pre regions live in their own PSUM bank
            HU = 2 * DU

            def win_mm(pw, half, wi):
                """All matmuls for window wi into pair-tile half."""
                base = half * PREB
                pbase = 2 * PREB + half * P
                for du in range(DU):
                    nc.tensor.matmul(
                        pw[:, base + du * 128:base + (du + 1) * 128],
                        xt[:, wi * XW + du * 128:wi * XW + (du + 1) * 128],
                        w_intra[:, du * 128:(du + 1) * 128],
                        start=True, stop=True, skip_group_check=True,
                    )
                # bias seed AFTER intras: start=True clears has_written
                # bank-wide; values already written stay valid
                nc.tensor.matmul(
                    pw[:, pbase:pbase + P], ones[:], brow[:],
                    start=True, stop=(NPRE == 0), skip_group_check=True,
                )
                for pu in range(NPRE):
                    nc.tensor.matmul(
                        pw[:, pbase:pbase + P],
                        xt[:, wi * XW + pu * 128:wi * XW + (pu + 1) * 128],
                        w_pre[:, pu * 128:(pu + 1) * 128],
                        start=False, stop=(pu == NPRE - 1),
                        skip_group_check=True,
                    )

            for pi in range(NP):
                pw = psW.tile([P, 2 * (PREB + P)], F32, name="pw", tag="pw")
                win_mm(pw, 0, 2 * pi)
                win_mm(pw, 1, 2 * pi + 1)
                # packed prefix totals (u >= first_u only) to SBUF
                pre_sb = mid.tile([P, HU * O], F32, name="pre_sb", tag="pre")
                nc.scalar.copy(
                    pre_sb[:].rearrange("p (h x) -> p h x", h=2),
                    pw[:, 2 * PREB:].rearrange(
                        "p (h x) -> p h x", h=2)[:, :, first_u * O:],
                )
                # eviction: out[(o, h, s)] = intra + prefix, fp16, one op/pair
                ot = outp.tile([P, 2 * YW], F16, name="ot", tag="ot")
                out4 = ot[:].rearrange(
                    "p (o hu v) -> p o hu v", o=O, hu=HU, v=V)
                in1 = pw[:, 0:2 * PREB].rearrange(
                    "p (hu v o) -> p o hu v", hu=HU, v=V, o=O)
                in2 = pre_sb[:].rearrange(
                    "p (hu o) -> p o hu", hu=HU, o=O
                ).unsqueeze(3).broadcast_to([P, O, HU, V])
                nc.vector.tensor_add(out4, in1, in2)
                eng = nc.scalar if pi % 2 == 0 else nc.sync
                eng.dma_start(
                    y_d.ap()[:, pi * 2 * YW:(pi + 1) * 2 * YW], ot[:])
    nc.compile()
    return nc


def _host_constants(weight, bias, n_discard, n_keep):
    Swin = n_discard + n_keep
    assert Swin == S and n_discard % V == 0
    first_u = n_discard // V
    DU = NU - first_u
    NPRE = DU - 1

    w = weight.reshape(O, C, n_keep).transpose(2, 1, 0).astype(np.float32)
    w_full = np.concatenate(
        [np.zeros((n_discard, C, O), np.float32), w], axis=0
    )  # (S, C, O)

    # w_intra[k=(c,vp), du*128 + v*16 + o] = w_full[u*8+vp, c, o] if vp<=v
    blk = np.stack([w_full[(first_u + du) * V:(first_u + du + 1) * V]
                    for du in range(DU)])          # (DU, V, C, O)
    tri = np.zeros((DU, C, V, V, O), np.float32)   # (du, c, vp, v, o)
    vp = np.arange(V)
    for v in range(V):
        tri[:, :, vp <= v, v, :] = blk.transpose(0, 2, 1, 3)[:, :, vp <= v]
    w_intra = tri.reshape(DU, C * V, V * O).transpose(1, 0, 2).reshape(
        P, DU * 128)

    # w_pre[k=(c,vp), pu*128 + ut*16 + o] = w_full[u*8+vp, c, o] if ut>u
    pre = np.zeros((max(NPRE, 1), C, V, NU, O), np.float32)
    for pu in range(NPRE):
        u = first_u + pu
        pre[pu, :, :, u + 1:, :] = blk[pu].transpose(1, 0, 2)[:, :, None, :]
    w_pre = pre.reshape(max(NPRE, 1), C * V, NU * O).transpose(1, 0, 2).reshape(
        P, max(NPRE, 1) * 128)

    bias32 = bias.astype(np.float32)
    consts = {
        "w_intra": np.ascontiguousarray(w_intra).astype(np.float16),
        "w_pre": np.ascontiguousarray(w_pre).astype(np.float16),
        "ones_k1": np.ones((1, P), np.float16),
        "biasrow": np.tile(bias32, NU)[None, :].astype(np.float16),
    }
    return consts, first_u


def _run(inputs, trace=False):
    x = np.asarray(inputs["x"], dtype=np.float32)
    weight = np.asarray(inputs["weight"], dtype=np.float32)
    bias = np.asarray(inputs["bias"], dtype=np.float32)
    n_discard = int(inputs["n_discard"])
    n_keep = int(inputs["n_keep"])
    assert x.shape == (B, C, T) and weight.shape == (O, C * n_keep)

    consts, first_u = _host_constants(weight, bias, n_discard, n_keep)
    DU = NU - first_u
    key = ("nc", first_u)
    if key not in _cache:
        _cache[key] = _build_nc(first_u)
    nc = _cache[key]

    # marshal x: fp16, drop s < n_discard, pre-transpose to
    # [(c,v), (st, w, u, tchunk p)]
    x16 = x.astype(np.float16)
    xr = x16.reshape(B, C, NST, P, NW, NU, V)[:, :, :, :, :, first_u:, :]
    # -> (B, C, V, NST, NW, DU, P) -> (B, (c v), (st w u p))
    xT = np.ascontiguousarray(xr.transpose(0, 1, 6, 2, 4, 5, 3)).reshape(
        B, C * V, NST * NW * DU * P)

    in_maps = []
    for b in range(B):
        m = dict(consts)
        m["xT"] = xT[b]
        in_maps.append(m)
    res = run_bass_kernel_spmd(nc, in_maps, list(range(B)), trace=trace)

    # assemble full output: device gives s >= n_discard, host fills bias
    KEEP = DU * V
    y = np.empty((B, O, T), np.float32)
    yw = y.reshape(B, O, NST, P, NW, S)
    yw[:, :, :, :, :, :n_discard] = bias.astype(np.float32)[
        None, :, None, None, None, None]
    NP = NST * NW // 2
    for b in range(B):
        yd = res.results[b]["y"].reshape(P, NP, O, 2, KEEP)
        arr = yd.transpose(2, 1, 3, 0, 4).reshape(O, NST, NW, P, KEEP)
        yw[b, :, :, :, :, n_discard:] = arr.transpose(0, 1, 3, 2, 4)
    return y, res


def kernel(**inputs):
    y, _ = _run(inputs, trace=False)
    return y
